# revision 37
# baseline (speedup 1.0000x reference)
"""Multi-head attention (softmax over query axis) on 8 TRN2 NeuronCores.

Data-parallel over batch: core b computes batch element b entirely locally
(B == n_cores == 8), so no collectives are needed.

Math (per batch element, x: [P, D]):
    qkv = x @ W_qkv ; q,k,v heads of dim DH=64
    dots = q @ k^T * SCALE              [h, P, P]
    A = softmax(dots, axis=-2)          (normalized over the QUERY axis i)
    out = (A @ v per head) @ W_out + b_out

Device strategy:
    xT [D, P] pre-transposed on host, W_q pre-scaled by SCALE, inputs bf16.
    dots_T[j, i] per head via 64-row-tiled PE head pairs (f32 psum).
    exp split across two engines: most [128j, 1024i] tiles take the native
    ScalarE Exp with accum_out row sums; three tiles per pair use a
    Schraudolph fast-exp on the DVE (y = x*A+B -> int16 -> bitcast bf16)
    plus an identity-with-accum for the sums. The reciprocal + V-row
    scaling are emitted per j-tile as soon as both heads' sums land, so the
    AV contraction can chase the exp stream (no per-pair softmax barrier).
    attn_out_T[dh, i] via 64-col-tiled PE head pairs accumulating over j,
    both heads in one psum tile; out-projection drains to bf16 and is
    DMA'd out bf16 (host upcasts + adds b_out).
"""
import numpy as np

import concourse.tile as tile
from concourse import bacc, mybir
from concourse.bass_utils import run_bass_kernel_spmd

B, P, D = 8, 1024, 512
H, DH = 8, 64
SCALE = DH ** -0.5
F32 = mybir.dt.float32
BF16 = mybir.dt.bfloat16
I16 = mybir.dt.int16
NCORES = 8

KT = D // 128        # 4 contraction k-tiles over D
PT = P // 128        # 8 p-tiles
NPAIR = H // 2       # 4 head pairs
IH = P // 512        # 2 i-halves (PSUM bank = 512 f32)

# Schraudolph fast-exp constants: exp(x) ~= bf16(bits = int16(x*SA + SB)).
SEXP_A = 128.0 / float(np.log(2.0))
SEXP_B = 128.0 * (127.0 - 0.04) + 0.25

# (jt, h) positions per pair whose exp runs on the DVE instead of ScalarE;
# spread across the pair so neither the DVE queue nor the AV j-walk bunches.
# All at h=1: the DVE op1 frees its dots psum ~3us late (queued behind
# proj CASTs), which gates the NEXT jt's same-head dots. At h1 the gated
# ACT is the second-next one ScalarE runs, so its intervening h0 ACTIVATE
# (1.4us) hides most of the op1 latency; at h0 the gap is fully exposed.
DVE_TILES = {(2, 1), (4, 1), (6, 1)}


def build():
    nc = bacc.Bacc(trn_type="TRN2")
    # inputs are host-packed so each tensor loads with few contiguous DMAs.
    # xT is ih-major ([128, IH, KT, 512]) so each i-half is one contiguous
    # transfer and the first projections can start after half the x data.
    xT_ext = nc.declare_dram_parameter("xT", [128, IH * KT * 512], BF16, isOutput=False)
    # wq/wk are c-slice-major ([128, NPAIR, KT, 128]) so the pair-0 slices
    # can be DMA'd ahead of the rest and dots can start ~10us earlier.
    wq_ext = nc.declare_dram_parameter("wq", [128, KT * D], BF16, isOutput=False)
    wk_ext = nc.declare_dram_parameter("wk", [128, KT * D], BF16, isOutput=False)
    wv_ext = nc.declare_dram_parameter("wv", [128, KT * D], BF16, isOutput=False)
    wo_ext = nc.declare_dram_parameter("wo", [128, KT * D], BF16, isOutput=False)
    out_ext = nc.declare_dram_parameter("out", [P, D], BF16, isOutput=True)

    with tile.TileContext(nc) as tc:
        with (
            tc.tile_pool(name="persist", bufs=1) as pp,
            # bufs=3: with 2 buffers, pair p's exp WARs pair p-1's AV j-walk
            # (still reading the same a_t/vp buffer mid-phase), chaining the
            # exp stream to AV filler pacing — the dominant ScalarE stall.
            # At 3 buffers the WAR partner is pair p-2, which is long done.
            tc.tile_pool(name="aT", bufs=3) as ap_,
            tc.tile_pool(name="vp", bufs=3) as vpp,
            tc.tile_pool(name="sums", bufs=3) as sp,
            tc.tile_pool(name="osb", bufs=4) as op_,
            tc.tile_pool(name="ps_main", bufs=2, space="PSUM") as ps_main,
            tc.tile_pool(name="ps_c", bufs=2, space="PSUM") as ps_c,
            tc.tile_pool(name="ps_d", bufs=1, space="PSUM") as ps_d,
        ):
            # ---- input DMA: 8 transfers spread over the 3 DMA-capable
            # engine queues (sync/scalar/gpsimd, ~140GB/s each) so the ramp
            # isn't serialized. First-needed data (xT halves, pair-0 q/k
            # slices) leads each queue; wv/wo/the remaining c-slices trail.
            xT = pp.tile([128, IH, KT, 512], BF16, name="xT", tag="xT")
            wq = pp.tile([128, NPAIR, KT, 128], BF16, name="wq", tag="wq")
            wk = pp.tile([128, NPAIR, KT, 128], BF16, name="wk", tag="wk")
            wv = pp.tile([128, KT, D], BF16, name="wv", tag="wv")
            wo = pp.tile([128, KT, D], BF16, name="wo", tag="wo")
            warm = pp.tile([128, 512], BF16, name="warm", tag="warm")
            CSL = KT * 128  # one c-slice of wq/wk per partition row
            nc.sync.dma_start(out=xT[:, 0], in_=xT_ext[:, : KT * 512])
            nc.gpsimd.dma_start(out=xT[:, 1], in_=xT_ext[:, KT * 512 :])
            nc.scalar.dma_start(out=wq[:, 0], in_=wq_ext[:, :CSL])
            nc.sync.dma_start(out=wk[:, 0], in_=wk_ext[:, :CSL])
            # memset on the DVE: a gpsimd memset between SWDGE triggers costs
            # a ~10us dge_drain on the gpsimd queue
            nc.vector.memset(warm, 0.0)
            nc.scalar.dma_start(out=wq[:, 1:], in_=wq_ext[:, CSL:])
            nc.sync.dma_start(out=wv, in_=wv_ext[:, :])
            nc.scalar.dma_start(out=wk[:, 1:], in_=wk_ext[:, CSL:])
            nc.gpsimd.dma_start(out=wo, in_=wo_ext[:, :])

            # PE warm-up during the DMA window: ~11 cold matmuls ~= 4.7us of
            # sustained PE activity (DMA data lands ~5.5us after trigger, so
            # the warmups bridge the whole wait) flips the HAM to 2.4GHz
            # right as the first real projection matmuls become ready
            for w_i in range(11):
                wps = ps_d.tile([128, 512], F32, name=f"ps_d{w_i % 2}", tag=f"ps_d{w_i % 2}")
                nc.tensor.matmul(out=wps, lhsT=warm[:, :128], rhs=warm,
                                 start=True, stop=True)

            # persistent activation storage
            qT = [pp.tile([128, P], BF16, name=f"qT{c}", tag=f"qT{c}") for c in range(NPAIR)]
            kTt = [pp.tile([128, P], BF16, name=f"kT{c}", tag=f"kT{c}") for c in range(NPAIR)]
            vt = [pp.tile([128, D], BF16, name=f"v{p}", tag=f"v{p}") for p in range(PT)]
            aoT = [pp.tile([128, P], BF16, name=f"aoT{c}", tag=f"aoT{c}") for c in range(NPAIR)]

            def proj_qk(w, ct, dst):
                """dst [128,P] = (x @ W)^T c-slice. Yields per i-half."""
                for ih in range(IH):
                    ps = ps_main.tile([128, 512], F32, name="ps_main", tag="ps_main")
                    for k in range(KT):
                        nc.tensor.matmul(
                            out=ps,
                            lhsT=w[:, ct, k, :],
                            rhs=xT[:, ih, k, :],
                            start=(k == 0), stop=(k == KT - 1),
                        )
                    nc.vector.tensor_copy(dst[:, ih * 512:(ih + 1) * 512], ps)
                    yield

            def proj_v(pt):
                """vt[pt] [128, D] = x p-tile @ W_v (bf16). Yields once."""
                ps = ps_main.tile([128, 512], F32, name="ps_main", tag="ps_main")
                for k in range(KT):
                    nc.tensor.matmul(
                        out=ps,
                        lhsT=xT[:, pt // 4, k, (pt % 4) * 128:(pt % 4) * 128 + 128],
                        rhs=wv[:, k, :],
                        start=(k == 0), stop=(k == KT - 1),
                    )
                nc.vector.tensor_copy(vt[pt], ps)
                yield

            pair_data = {}

            def pe_filler(n):
                """n dummy N=512 matmuls into the (idle) ps_main pool.

                The HAM re-throttles the PE to 1.2GHz when a ~3.4us activity
                window sees too much idle (observed threshold: phases at
                <=55% PE-busy go cold and stay cold; >=74% stay at 2.4GHz).
                The attention-only phases are elementwise-bound with the PE
                at ~37-55%, so they get padded with dummy matmuls.
                """
                for _ in range(n):
                    wps = ps_main.tile([128, 512], F32, name="ps_main", tag="ps_main")
                    nc.tensor.matmul(out=wps, lhsT=warm[:, :128], rhs=warm,
                                     start=True, stop=True)

            def attn_pair(pr, dummies=0):
                """dots + exp + row sums for head pair pr. Yields per (jt, h).

                Most tiles: ScalarE Exp with accum_out sums. DVE_TILES:
                Schraudolph fast-exp on DVE + identity-with-accum sums.
                After both heads of a j-tile are done, the reciprocal and
                V-row scaling for that j-tile are emitted immediately, so
                the AV contraction can start before the whole pair's exp
                stream finishes (no per-pair softmax-sum barrier).
                """
                a_t = [[ap_.tile([128, P], BF16, name=f"a{h}_{jt}", tag=f"a{h}_{jt}")
                        for jt in range(PT)] for h in range(2)]
                sums = sp.tile([128, 2, PT], F32, name="sums", tag="sums")
                rr = sp.tile([128, 2, PT], F32, name="recip", tag="recip")
                vp = [vpp.tile([128, 128], BF16, name=f"vp{jt}", tag=f"vp{jt}")
                      for jt in range(PT)]
                pair_data[pr] = (a_t, vp)
                for jt in range(PT):
                    for h in range(2):
                        hp = slice(h * 64, (h + 1) * 64)
                        ps = ps_c.tile([128, P], F32, name="ps_c", tag="ps_c")
                        for ih in range(IH):
                            nc.tensor.matmul(
                                out=ps[:, ih * 512:(ih + 1) * 512],
                                lhsT=kTt[pr][hp, jt * 128:(jt + 1) * 128],
                                rhs=qT[pr][hp, ih * 512:(ih + 1) * 512],
                                start=True, stop=True,
                                tile_position=(h * 64, 0),
                            )
                        if (jt, h) in DVE_TILES:
                            nc.vector.tensor_scalar(
                                out=a_t[h][jt].bitcast(I16),
                                in0=ps,
                                scalar1=SEXP_A,
                                scalar2=SEXP_B,
                                op0=mybir.AluOpType.mult,
                                op1=mybir.AluOpType.add,
                            )
                            nc.vector.tensor_scalar(
                                out=a_t[h][jt],
                                in0=a_t[h][jt],
                                scalar1=1.0,
                                scalar2=None,
                                op0=mybir.AluOpType.mult,
                                op1=mybir.AluOpType.add,
                                accum_out=sums[:, h, jt:jt + 1],
                            )
                        else:
                            nc.scalar.activation(
                                out=a_t[h][jt],
                                in_=ps,
                                func=mybir.ActivationFunctionType.Exp,
                                accum_out=sums[:, h, jt:jt + 1],
                            )
                        if h == 1:
                            nc.vector.reciprocal(
                                rr[:, :, jt:jt + 1], sums[:, :, jt:jt + 1]
                            )
                            for hh in range(2):
                                hc = (2 * pr + hh) * 64
                                # NOTE: keep this on the DVE — gpsimd takes
                                # ~1.15us per op (Q7 dispatch overhead) and
                                # this sits on the critical path to AV
                                nc.vector.tensor_scalar_mul(
                                    vp[jt][:, hh * 64:(hh + 1) * 64],
                                    vt[jt][:, hc:hc + 64],
                                    rr[:, hh, jt:jt + 1],
                                )
                            # dummies after both heads' dots so they never
                            # delay the exp stream
                            pe_filler(dummies)
                        yield

            def attn_av_ih(pr, ih):
                """contract A^T with V' for one i-half of pair pr -> aoT[pr].

                Both heads accumulate into one [128,512] psum tile (disjoint
                partition halves), so the drain is a single CAST.
                """
                a_t, vp = pair_data[pr]
                psd = ps_d.tile([128, 512], F32,
                                name=f"ps_d{ih % 2}", tag=f"ps_d{ih % 2}")
                for jt in range(PT):
                    for h in range(2):
                        nc.tensor.matmul(
                            out=psd[h * 64:(h + 1) * 64, :],
                            lhsT=vp[jt][:, h * 64:(h + 1) * 64],
                            rhs=a_t[h][jt][:, ih * 512:(ih + 1) * 512],
                            start=(jt == 0), stop=(jt == PT - 1),
                            tile_position=(0, h * 64),
                            skip_group_check=True,
                        )
                    if jt % 2 == 1:
                        yield
                nc.vector.tensor_copy(
                    aoT[pr][:, ih * 512:(ih + 1) * 512], psd
                )
                yield

            def attn_av(pr):
                yield from attn_av_ih(pr, 0)
                yield from attn_av_ih(pr, 1)

            def chain(*gens):
                for g in gens:
                    yield from g

            def interleave(main, filler, ms=2, fs=2, filler_first=False):
                """Emit ms units of main, then fs units of filler, repeating.

                filler_first puts the filler (usually the previous pair's AV,
                whose dependencies are already satisfied) ahead of the new
                pair's dots in the in-order PE queue each round.
                """
                order = ((filler, fs), (main, ms)) if filler_first \
                    else ((main, ms), (filler, fs))
                while True:
                    done = 0
                    for g, n in order:
                        try:
                            for _ in range(n):
                                next(g)
                        except StopIteration:
                            done += 1
                    if done == 2:
                        return

            def out_proj(pts):
                for pt in pts:
                    ps = ps_main.tile([128, 512], F32, name="ps_main", tag="ps_main")
                    for ct in range(KT):
                        nc.tensor.matmul(
                            out=ps,
                            lhsT=aoT[ct][:, pt * 128:(pt + 1) * 128],
                            rhs=wo[:, ct, :],
                            start=(ct == 0), stop=(ct == KT - 1),
                        )
                    ot = op_.tile([128, 512], BF16, name="osb", tag="osb")
                    nc.vector.tensor_copy(ot, ps)
                    eng = nc.sync if pt % 2 == 0 else nc.scalar
                    eng.dma_start(out=out_ext[pt * 128:(pt + 1) * 128, :], in_=ot)
                    yield

            for g in chain(proj_qk(wq, 0, qT[0]), proj_qk(wk, 0, kTt[0])):
                pass
            # filler_first so each proj_v(jt) is emitted before the pair-0
            # vp-scaling unit that reads vt[jt]. Pair phases 1-2 are
            # PE-starved (elementwise-bound), so they get dummy-matmul
            # padding to keep the HAM from re-throttling the PE clock.
            # ms=2/fs=1: two dots tiles per filler unit. The exp span is
            # production-rate-limited at 1:1 (one tile per ~1.7us round vs
            # ScalarE's 1.4us/tile consumption); a production surplus lets
            # the exp stream run back-to-back and the filler drains inside
            # the PE's psum-free stall slack.
            interleave(
                attn_pair(0),
                chain(*[proj_v(pt) for pt in range(PT)],
                      proj_qk(wq, 1, qT[1]), proj_qk(wk, 1, kTt[1]),
                      proj_qk(wq, 2, qT[2]), proj_qk(wk, 2, kTt[2])),
                ms=2, fs=2, filler_first=True,
            )
            interleave(
                attn_pair(1, dummies=2),
                chain(attn_av(0),
                      proj_qk(wq, 3, qT[3]), proj_qk(wk, 3, kTt[3])),
                ms=2, fs=2, filler_first=True,
            )
            interleave(attn_pair(2, dummies=4), attn_av(1),
                       ms=2, fs=2, filler_first=True)
            # pair 3: AV(2), the pair-3 AV halves and the out-projection all
            # pipeline into the exp-supply rounds. ms=3/fs=2 keeps every
            # pair-3 AV j-tile behind its dots in the in-order PE queue
            # (emitting an AV j-tile before its dots would deadlock the PE).
            interleave(
                attn_pair(3),
                chain(attn_av(2),
                      attn_av_ih(3, 0),
                      out_proj(range(4)),
                      attn_av_ih(3, 1),
                      out_proj(range(4, PT))),
                ms=3, fs=2, filler_first=True,
            )


    nc.finalize()
    return nc


_NC = None


def _get_nc():
    global _NC
    if _NC is None:
        _NC = build()
    return _NC


def run(x, W_qkv, W_out, b_out, trace=False, tmpdir=None):
    import ml_dtypes

    x = np.asarray(x, dtype=np.float32)
    W_qkv = np.asarray(W_qkv, dtype=np.float32)
    W_out = np.asarray(W_out, dtype=np.float32)
    b_out = np.asarray(b_out, dtype=np.float32)

    bf = ml_dtypes.bfloat16

    def pack(a):
        # [KT*128, W] -> [128, KT*W] (k-tile-major per partition row)
        w = a.shape[1]
        return np.ascontiguousarray(
            a.reshape(KT, 128, w).transpose(1, 0, 2).reshape(128, KT * w)
        ).astype(bf)

    def pack_x(a):
        # a = x[b].T [D, P] -> [128, IH*KT*512] (i-half-major, then k-tile)
        t = a.reshape(KT, 128, IH, 512)
        return np.ascontiguousarray(
            t.transpose(1, 2, 0, 3).reshape(128, IH * KT * 512)
        ).astype(bf)

    def pack_c(a):
        # [KT*128, D] -> [128, NPAIR*KT*128] (c-slice-major, then k-tile)
        t = a.reshape(KT, 128, H // 2, 128)
        return np.ascontiguousarray(
            t.transpose(1, 2, 0, 3).reshape(128, (H // 2) * KT * 128)
        ).astype(bf)

    wq_h = pack_c(W_qkv[:, :D] * np.float32(SCALE))
    wk_h = pack_c(W_qkv[:, D:2 * D])
    wv_h = pack(W_qkv[:, 2 * D:])
    wo_h = pack(W_out)
    in_maps = [
        {
            "xT": pack_x(np.ascontiguousarray(x[b].T)),
            "wq": wq_h, "wk": wk_h, "wv": wv_h, "wo": wo_h,
        }
        for b in range(NCORES)
    ]
    nc = _get_nc()
    res = run_bass_kernel_spmd(
        nc, in_maps, core_ids=list(range(NCORES)), trace=trace, tmpdir=tmpdir
    )
    out = np.stack(
        [res.results[b]["out"].astype(np.float32) for b in range(NCORES)], axis=0
    )
    out = out + b_out[None, None, :]
    return out.astype(np.float32), res


def kernel(x, W_qkv, W_out, b_out):
    out, _ = run(x, W_qkv, W_out, b_out, trace=False)
    return out



# revision 41
# speedup vs baseline: 1.1576x; 1.1576x over previous
"""Multi-head attention (softmax over query axis) on 8 TRN2 NeuronCores.

Data-parallel over batch: core b computes batch element b entirely locally
(B == n_cores == 8), so no collectives are needed.

Math (per batch element, x: [P, D]):
    qkv = x @ W_qkv ; q,k,v heads of dim DH=64
    dots = q @ k^T * SCALE              [h, P, P]
    A = softmax(dots, axis=-2)          (normalized over the QUERY axis i)
    out = (A @ v per head) @ W_out + b_out

Device strategy:
    xT [D, P] pre-transposed on host, W_q pre-scaled by SCALE, inputs bf16.
    dots_T[j, i] per head via 64-row-tiled PE head pairs (f32 psum).
    exp split across two engines: most [128j, 1024i] tiles take the native
    ScalarE Exp with accum_out row sums; three tiles per pair use a
    Schraudolph fast-exp on the DVE (y = x*A+B -> int16 -> bitcast bf16)
    plus an identity-with-accum for the sums. The reciprocal + V-row
    scaling are emitted per j-tile as soon as both heads' sums land, so the
    AV contraction can chase the exp stream (no per-pair softmax barrier).
    attn_out_T[dh, i] via 64-col-tiled PE head pairs accumulating over j,
    both heads in one psum tile; out-projection drains to bf16 and is
    DMA'd out bf16 (host upcasts + adds b_out).
"""
import numpy as np

import concourse.tile as tile
from concourse import bacc, mybir
from concourse.bass_utils import run_bass_kernel_spmd

B, P, D = 8, 1024, 512
H, DH = 8, 64
SCALE = DH ** -0.5
F32 = mybir.dt.float32
BF16 = mybir.dt.bfloat16
I16 = mybir.dt.int16
NCORES = 8

KT = D // 128        # 4 contraction k-tiles over D
PT = P // 128        # 8 p-tiles
NPAIR = H // 2       # 4 head pairs
IH = P // 512        # 2 i-halves (PSUM bank = 512 f32)

# Schraudolph fast-exp constants: exp(x) ~= bf16(bits = int16(x*SA + SB)).
SEXP_A = 128.0 / float(np.log(2.0))
SEXP_B = 128.0 * (127.0 - 0.04) + 0.25

# (jt, h) positions per pair whose exp runs on the DVE instead of ScalarE;
# spread across the pair so neither the DVE queue nor the AV j-walk bunches.
DVE_TILES = {(2, 0), (4, 1), (6, 0)}


def build():
    nc = bacc.Bacc(trn_type="TRN2")
    # inputs are host-packed so each tensor loads with few contiguous DMAs.
    # xT is ih-major ([128, IH, KT, 512]) so each i-half is one contiguous
    # transfer and the first projections can start after half the x data.
    xT_ext = nc.declare_dram_parameter("xT", [128, IH * KT * 512], BF16, isOutput=False)
    # wq/wk are c-slice-major ([128, NPAIR, KT, 128]) so the pair-0 slices
    # can be DMA'd ahead of the rest and dots can start ~10us earlier.
    wq_ext = nc.declare_dram_parameter("wq", [128, KT * D], BF16, isOutput=False)
    wk_ext = nc.declare_dram_parameter("wk", [128, KT * D], BF16, isOutput=False)
    wv_ext = nc.declare_dram_parameter("wv", [128, KT * D], BF16, isOutput=False)
    wo_ext = nc.declare_dram_parameter("wo", [128, KT * D], BF16, isOutput=False)
    out_ext = nc.declare_dram_parameter("out", [P, D], BF16, isOutput=True)

    with tile.TileContext(nc) as tc:
        with (
            tc.tile_pool(name="persist", bufs=1) as pp,
            # bufs=3: with 2 buffers, pair p's exp WARs pair p-1's AV j-walk
            # (still reading the same a_t/vp buffer mid-phase), chaining the
            # exp stream to AV filler pacing — the dominant ScalarE stall.
            # At 3 buffers the WAR partner is pair p-2, which is long done.
            tc.tile_pool(name="aT", bufs=3) as ap_,
            tc.tile_pool(name="vp", bufs=3) as vpp,
            tc.tile_pool(name="sums", bufs=3) as sp,
            tc.tile_pool(name="osb", bufs=4) as op_,
            tc.tile_pool(name="ps_main", bufs=2, space="PSUM") as ps_main,
            tc.tile_pool(name="ps_c", bufs=2, space="PSUM") as ps_c,
            tc.tile_pool(name="ps_d", bufs=1, space="PSUM") as ps_d,
        ):
            # ---- input DMA: 8 transfers spread over the 3 DMA-capable
            # engine queues (sync/scalar/gpsimd, ~140GB/s each) so the ramp
            # isn't serialized. First-needed data (xT halves, pair-0 q/k
            # slices) leads each queue; wv/wo/the remaining c-slices trail.
            xT = pp.tile([128, IH, KT, 512], BF16, name="xT", tag="xT")
            wq = pp.tile([128, NPAIR, KT, 128], BF16, name="wq", tag="wq")
            wk = pp.tile([128, NPAIR, KT, 128], BF16, name="wk", tag="wk")
            wv = pp.tile([128, KT, D], BF16, name="wv", tag="wv")
            wo = pp.tile([128, KT, D], BF16, name="wo", tag="wo")
            warm = pp.tile([128, 512], BF16, name="warm", tag="warm")
            CSL = KT * 128  # one c-slice of wq/wk per partition row
            nc.sync.dma_start(out=xT[:, 0], in_=xT_ext[:, : KT * 512])
            nc.gpsimd.dma_start(out=xT[:, 1], in_=xT_ext[:, KT * 512 :])
            nc.scalar.dma_start(out=wq[:, 0], in_=wq_ext[:, :CSL])
            nc.sync.dma_start(out=wk[:, 0], in_=wk_ext[:, :CSL])
            # memset on the DVE: a gpsimd memset between SWDGE triggers costs
            # a ~10us dge_drain on the gpsimd queue
            nc.vector.memset(warm, 0.0)
            nc.scalar.dma_start(out=wq[:, 1:], in_=wq_ext[:, CSL:])
            nc.sync.dma_start(out=wv, in_=wv_ext[:, :])
            nc.scalar.dma_start(out=wk[:, 1:], in_=wk_ext[:, CSL:])
            nc.gpsimd.dma_start(out=wo, in_=wo_ext[:, :])

            # PE warm-up during the DMA window: ~11 cold matmuls ~= 4.7us of
            # sustained PE activity (DMA data lands ~5.5us after trigger, so
            # the warmups bridge the whole wait) flips the HAM to 2.4GHz
            # right as the first real projection matmuls become ready
            for w_i in range(11):
                wps = ps_d.tile([128, 512], F32, name=f"ps_d{w_i % 2}", tag=f"ps_d{w_i % 2}")
                nc.tensor.matmul(out=wps, lhsT=warm[:, :128], rhs=warm,
                                 start=True, stop=True)

            # persistent activation storage
            qT = [pp.tile([128, P], BF16, name=f"qT{c}", tag=f"qT{c}") for c in range(NPAIR)]
            kTt = [pp.tile([128, P], BF16, name=f"kT{c}", tag=f"kT{c}") for c in range(NPAIR)]
            vt = [pp.tile([128, D], BF16, name=f"v{p}", tag=f"v{p}") for p in range(PT)]
            aoT = [pp.tile([128, P], BF16, name=f"aoT{c}", tag=f"aoT{c}") for c in range(NPAIR)]

            # Deferred proj drains: interleaved filler units append their
            # psum->sbuf CAST here instead of emitting it inline; the next
            # (jt,h1) attention unit flushes the list AFTER its DVE exp ops.
            # This puts the Schraudolph op1/op2 (which free dots psum tiles)
            # AHEAD of the proj CASTs in the DVE queue — op1 queueing ~1.4us
            # behind two CASTs per round was the dominant remaining ScalarE
            # stall. At most 2 drains pend (2 filler units/round, ps_main
            # bufs=2), and the flush is emitted before the next round's
            # filler allocations, so the WAR chain stays intact.
            pend_drains = []

            def flush_drains():
                while pend_drains:
                    dsl, psrc = pend_drains.pop(0)
                    nc.vector.tensor_copy(dsl, psrc)

            def proj_qk(w, ct, dst, defer=False):
                """dst [128,P] = (x @ W)^T c-slice. Yields per i-half."""
                for ih in range(IH):
                    ps = ps_main.tile([128, 512], F32, name="ps_main", tag="ps_main")
                    for k in range(KT):
                        nc.tensor.matmul(
                            out=ps,
                            lhsT=w[:, ct, k, :],
                            rhs=xT[:, ih, k, :],
                            start=(k == 0), stop=(k == KT - 1),
                        )
                    if defer:
                        pend_drains.append((dst[:, ih * 512:(ih + 1) * 512], ps))
                    else:
                        nc.vector.tensor_copy(dst[:, ih * 512:(ih + 1) * 512], ps)
                    yield

            def proj_v(pt, defer=False):
                """vt[pt] [128, D] = x p-tile @ W_v (bf16). Yields once."""
                ps = ps_main.tile([128, 512], F32, name="ps_main", tag="ps_main")
                for k in range(KT):
                    nc.tensor.matmul(
                        out=ps,
                        lhsT=xT[:, pt // 4, k, (pt % 4) * 128:(pt % 4) * 128 + 128],
                        rhs=wv[:, k, :],
                        start=(k == 0), stop=(k == KT - 1),
                    )
                if defer:
                    pend_drains.append((vt[pt], ps))
                else:
                    nc.vector.tensor_copy(vt[pt], ps)
                yield

            pair_data = {}

            def pe_filler(n):
                """n dummy N=512 matmuls into the (idle) ps_main pool.

                The HAM re-throttles the PE to 1.2GHz when a ~3.4us activity
                window sees too much idle (observed threshold: phases at
                <=55% PE-busy go cold and stay cold; >=74% stay at 2.4GHz).
                The attention-only phases are elementwise-bound with the PE
                at ~37-55%, so they get padded with dummy matmuls.
                """
                for _ in range(n):
                    wps = ps_main.tile([128, 512], F32, name="ps_main", tag="ps_main")
                    nc.tensor.matmul(out=wps, lhsT=warm[:, :128], rhs=warm,
                                     start=True, stop=True)

            def attn_pair(pr, dummies=0):
                """dots + exp + row sums for head pair pr. Yields per (jt, h).

                Most tiles: ScalarE Exp with accum_out sums. DVE_TILES:
                Schraudolph fast-exp on DVE + identity-with-accum sums.
                After both heads of a j-tile are done, the reciprocal and
                V-row scaling for that j-tile are emitted immediately, so
                the AV contraction can start before the whole pair's exp
                stream finishes (no per-pair softmax-sum barrier).
                """
                a_t = [[ap_.tile([128, P], BF16, name=f"a{h}_{jt}", tag=f"a{h}_{jt}")
                        for jt in range(PT)] for h in range(2)]
                sums = sp.tile([128, 2, PT], F32, name="sums", tag="sums")
                rr = sp.tile([128, 2, PT], F32, name="recip", tag="recip")
                vp = [vpp.tile([128, 128], BF16, name=f"vp{jt}", tag=f"vp{jt}")
                      for jt in range(PT)]
                pair_data[pr] = (a_t, vp)
                for jt in range(PT):
                    for h in range(2):
                        hp = slice(h * 64, (h + 1) * 64)
                        ps = ps_c.tile([128, P], F32, name="ps_c", tag="ps_c")
                        for ih in range(IH):
                            nc.tensor.matmul(
                                out=ps[:, ih * 512:(ih + 1) * 512],
                                lhsT=kTt[pr][hp, jt * 128:(jt + 1) * 128],
                                rhs=qT[pr][hp, ih * 512:(ih + 1) * 512],
                                start=True, stop=True,
                                tile_position=(h * 64, 0),
                            )
                        if (jt, h) in DVE_TILES:
                            nc.vector.tensor_scalar(
                                out=a_t[h][jt].bitcast(I16),
                                in0=ps,
                                scalar1=SEXP_A,
                                scalar2=SEXP_B,
                                op0=mybir.AluOpType.mult,
                                op1=mybir.AluOpType.add,
                            )
                            nc.vector.tensor_scalar(
                                out=a_t[h][jt],
                                in0=a_t[h][jt],
                                scalar1=1.0,
                                scalar2=None,
                                op0=mybir.AluOpType.mult,
                                op1=mybir.AluOpType.add,
                                accum_out=sums[:, h, jt:jt + 1],
                            )
                        else:
                            nc.scalar.activation(
                                out=a_t[h][jt],
                                in_=ps,
                                func=mybir.ActivationFunctionType.Exp,
                                accum_out=sums[:, h, jt:jt + 1],
                            )
                        if h == 1:
                            # flush deferred proj drains first: their CASTs
                            # land on the DVE queue after this unit's exp
                            # ops but before recip/vscale (and before the
                            # vp-scale below reads any vt drained here)
                            flush_drains()
                            nc.vector.reciprocal(
                                rr[:, :, jt:jt + 1], sums[:, :, jt:jt + 1]
                            )
                            for hh in range(2):
                                hc = (2 * pr + hh) * 64
                                # NOTE: keep this on the DVE — gpsimd takes
                                # ~1.15us per op (Q7 dispatch overhead) and
                                # this sits on the critical path to AV
                                nc.vector.tensor_scalar_mul(
                                    vp[jt][:, hh * 64:(hh + 1) * 64],
                                    vt[jt][:, hc:hc + 64],
                                    rr[:, hh, jt:jt + 1],
                                )
                            # dummies after both heads' dots so they never
                            # delay the exp stream
                            pe_filler(dummies)
                        yield

            def attn_av_ih(pr, ih):
                """contract A^T with V' for one i-half of pair pr -> aoT[pr].

                Both heads accumulate into one [128,512] psum tile (disjoint
                partition halves), so the drain is a single CAST.
                """
                a_t, vp = pair_data[pr]
                psd = ps_d.tile([128, 512], F32,
                                name=f"ps_d{ih % 2}", tag=f"ps_d{ih % 2}")
                for jt in range(PT):
                    for h in range(2):
                        nc.tensor.matmul(
                            out=psd[h * 64:(h + 1) * 64, :],
                            lhsT=vp[jt][:, h * 64:(h + 1) * 64],
                            rhs=a_t[h][jt][:, ih * 512:(ih + 1) * 512],
                            start=(jt == 0), stop=(jt == PT - 1),
                            tile_position=(0, h * 64),
                            skip_group_check=True,
                        )
                    if jt % 2 == 1:
                        yield
                nc.vector.tensor_copy(
                    aoT[pr][:, ih * 512:(ih + 1) * 512], psd
                )
                yield

            def attn_av(pr):
                yield from attn_av_ih(pr, 0)
                yield from attn_av_ih(pr, 1)

            def chain(*gens):
                for g in gens:
                    yield from g

            def interleave(main, filler, ms=2, fs=2, filler_first=False):
                """Emit ms units of main, then fs units of filler, repeating.

                filler_first puts the filler (usually the previous pair's AV,
                whose dependencies are already satisfied) ahead of the new
                pair's dots in the in-order PE queue each round.
                """
                order = ((filler, fs), (main, ms)) if filler_first \
                    else ((main, ms), (filler, fs))
                while True:
                    done = 0
                    for g, n in order:
                        try:
                            for _ in range(n):
                                next(g)
                        except StopIteration:
                            done += 1
                    if done == 2:
                        return

            def out_proj(pts):
                for pt in pts:
                    ps = ps_main.tile([128, 512], F32, name="ps_main", tag="ps_main")
                    for ct in range(KT):
                        nc.tensor.matmul(
                            out=ps,
                            lhsT=aoT[ct][:, pt * 128:(pt + 1) * 128],
                            rhs=wo[:, ct, :],
                            start=(ct == 0), stop=(ct == KT - 1),
                        )
                    ot = op_.tile([128, 512], BF16, name="osb", tag="osb")
                    nc.vector.tensor_copy(ot, ps)
                    eng = nc.sync if pt % 2 == 0 else nc.scalar
                    eng.dma_start(out=out_ext[pt * 128:(pt + 1) * 128, :], in_=ot)
                    yield

            for g in chain(proj_qk(wq, 0, qT[0]), proj_qk(wk, 0, kTt[0])):
                pass
            # filler_first so each proj_v(jt) is emitted before the pair-0
            # vp-scaling unit that reads vt[jt]. Pair phases 1-2 are
            # PE-starved (elementwise-bound), so they get dummy-matmul
            # padding to keep the HAM from re-throttling the PE clock.
            # ms=2/fs=1: two dots tiles per filler unit. The exp span is
            # production-rate-limited at 1:1 (one tile per ~1.7us round vs
            # ScalarE's 1.4us/tile consumption); a production surplus lets
            # the exp stream run back-to-back and the filler drains inside
            # the PE's psum-free stall slack.
            interleave(
                attn_pair(0),
                chain(*[proj_v(pt, defer=True) for pt in range(PT)],
                      proj_qk(wq, 1, qT[1], defer=True),
                      proj_qk(wk, 1, kTt[1], defer=True),
                      proj_qk(wq, 2, qT[2], defer=True),
                      proj_qk(wk, 2, kTt[2], defer=True)),
                ms=2, fs=2, filler_first=True,
            )
            interleave(
                attn_pair(1, dummies=2),
                chain(attn_av(0),
                      proj_qk(wq, 3, qT[3], defer=True),
                      proj_qk(wk, 3, kTt[3], defer=True)),
                ms=2, fs=2, filler_first=True,
            )
            interleave(attn_pair(2, dummies=4), attn_av(1),
                       ms=2, fs=2, filler_first=True)
            # pair 3: AV(2), the pair-3 AV halves and the out-projection all
            # pipeline into the exp-supply rounds. ms=3/fs=2 keeps every
            # pair-3 AV j-tile behind its dots in the in-order PE queue
            # (emitting an AV j-tile before its dots would deadlock the PE).
            interleave(
                attn_pair(3),
                chain(attn_av(2),
                      attn_av_ih(3, 0),
                      out_proj(range(4)),
                      attn_av_ih(3, 1),
                      out_proj(range(4, PT))),
                ms=3, fs=2, filler_first=True,
            )


    nc.finalize()
    return nc


_NC = None


def _get_nc():
    global _NC
    if _NC is None:
        _NC = build()
    return _NC


def run(x, W_qkv, W_out, b_out, trace=False, tmpdir=None):
    import ml_dtypes

    x = np.asarray(x, dtype=np.float32)
    W_qkv = np.asarray(W_qkv, dtype=np.float32)
    W_out = np.asarray(W_out, dtype=np.float32)
    b_out = np.asarray(b_out, dtype=np.float32)

    bf = ml_dtypes.bfloat16

    def pack(a):
        # [KT*128, W] -> [128, KT*W] (k-tile-major per partition row)
        w = a.shape[1]
        return np.ascontiguousarray(
            a.reshape(KT, 128, w).transpose(1, 0, 2).reshape(128, KT * w)
        ).astype(bf)

    def pack_x(a):
        # a = x[b].T [D, P] -> [128, IH*KT*512] (i-half-major, then k-tile)
        t = a.reshape(KT, 128, IH, 512)
        return np.ascontiguousarray(
            t.transpose(1, 2, 0, 3).reshape(128, IH * KT * 512)
        ).astype(bf)

    def pack_c(a):
        # [KT*128, D] -> [128, NPAIR*KT*128] (c-slice-major, then k-tile)
        t = a.reshape(KT, 128, H // 2, 128)
        return np.ascontiguousarray(
            t.transpose(1, 2, 0, 3).reshape(128, (H // 2) * KT * 128)
        ).astype(bf)

    wq_h = pack_c(W_qkv[:, :D] * np.float32(SCALE))
    wk_h = pack_c(W_qkv[:, D:2 * D])
    wv_h = pack(W_qkv[:, 2 * D:])
    wo_h = pack(W_out)
    in_maps = [
        {
            "xT": pack_x(np.ascontiguousarray(x[b].T)),
            "wq": wq_h, "wk": wk_h, "wv": wv_h, "wo": wo_h,
        }
        for b in range(NCORES)
    ]
    nc = _get_nc()
    res = run_bass_kernel_spmd(
        nc, in_maps, core_ids=list(range(NCORES)), trace=trace, tmpdir=tmpdir
    )
    out = np.stack(
        [res.results[b]["out"].astype(np.float32) for b in range(NCORES)], axis=0
    )
    out = out + b_out[None, None, :]
    return out.astype(np.float32), res


def kernel(x, W_qkv, W_out, b_out):
    out, _ = run(x, W_qkv, W_out, b_out, trace=False)
    return out



# revision 42
# speedup vs baseline: 1.1731x; 1.0133x over previous
"""Multi-head attention (softmax over query axis) on 8 TRN2 NeuronCores.

Data-parallel over batch: core b computes batch element b entirely locally
(B == n_cores == 8), so no collectives are needed.

Math (per batch element, x: [P, D]):
    qkv = x @ W_qkv ; q,k,v heads of dim DH=64
    dots = q @ k^T * SCALE              [h, P, P]
    A = softmax(dots, axis=-2)          (normalized over the QUERY axis i)
    out = (A @ v per head) @ W_out + b_out

Device strategy:
    xT [D, P] pre-transposed on host, W_q pre-scaled by SCALE, inputs bf16.
    dots_T[j, i] per head via 64-row-tiled PE head pairs (f32 psum).
    exp split across two engines: most [128j, 1024i] tiles take the native
    ScalarE Exp with accum_out row sums; three tiles per pair use a
    Schraudolph fast-exp on the DVE (y = x*A+B -> int16 -> bitcast bf16)
    plus an identity-with-accum for the sums. The reciprocal + V-row
    scaling are emitted per j-tile as soon as both heads' sums land, so the
    AV contraction can chase the exp stream (no per-pair softmax barrier).
    attn_out_T[dh, i] via 64-col-tiled PE head pairs accumulating over j,
    both heads in one psum tile; out-projection drains to bf16 and is
    DMA'd out bf16 (host upcasts + adds b_out).
"""
import numpy as np

import concourse.tile as tile
from concourse import bacc, mybir
from concourse.bass_utils import run_bass_kernel_spmd

B, P, D = 8, 1024, 512
H, DH = 8, 64
SCALE = DH ** -0.5
F32 = mybir.dt.float32
BF16 = mybir.dt.bfloat16
I16 = mybir.dt.int16
NCORES = 8

KT = D // 128        # 4 contraction k-tiles over D
PT = P // 128        # 8 p-tiles
NPAIR = H // 2       # 4 head pairs
IH = P // 512        # 2 i-halves (PSUM bank = 512 f32)

# Schraudolph fast-exp constants: exp(x) ~= bf16(bits = int16(x*SA + SB)).
SEXP_A = 128.0 / float(np.log(2.0))
SEXP_B = 128.0 * (127.0 - 0.04) + 0.25

# (jt, h) positions per pair whose exp runs on the DVE instead of ScalarE;
# spread across the pair so neither the DVE queue nor the AV j-walk bunches.
DVE_TILES = {(2, 0), (4, 1), (6, 0)}


def build():
    nc = bacc.Bacc(trn_type="TRN2")
    # inputs are host-packed so each tensor loads with few contiguous DMAs.
    # xT is ih-major ([128, IH, KT, 512]) so each i-half is one contiguous
    # transfer and the first projections can start after half the x data.
    xT_ext = nc.declare_dram_parameter("xT", [128, IH * KT * 512], BF16, isOutput=False)
    # wq/wk are c-slice-major ([128, NPAIR, KT, 128]) so the pair-0 slices
    # can be DMA'd ahead of the rest and dots can start ~10us earlier.
    wq_ext = nc.declare_dram_parameter("wq", [128, KT * D], BF16, isOutput=False)
    wk_ext = nc.declare_dram_parameter("wk", [128, KT * D], BF16, isOutput=False)
    wv_ext = nc.declare_dram_parameter("wv", [128, KT * D], BF16, isOutput=False)
    wo_ext = nc.declare_dram_parameter("wo", [128, KT * D], BF16, isOutput=False)
    out_ext = nc.declare_dram_parameter("out", [P, D], BF16, isOutput=True)

    with tile.TileContext(nc) as tc:
        with (
            tc.tile_pool(name="persist", bufs=1) as pp,
            # bufs=3: with 2 buffers, pair p's exp WARs pair p-1's AV j-walk
            # (still reading the same a_t/vp buffer mid-phase), chaining the
            # exp stream to AV filler pacing — the dominant ScalarE stall.
            # At 3 buffers the WAR partner is pair p-2, which is long done.
            tc.tile_pool(name="aT", bufs=3) as ap_,
            tc.tile_pool(name="vp", bufs=3) as vpp,
            tc.tile_pool(name="sums", bufs=3) as sp,
            tc.tile_pool(name="osb", bufs=4) as op_,
            tc.tile_pool(name="ps_main", bufs=2, space="PSUM") as ps_main,
            tc.tile_pool(name="ps_c", bufs=2, space="PSUM") as ps_c,
            tc.tile_pool(name="ps_d", bufs=1, space="PSUM") as ps_d,
        ):
            # ---- input DMA: 8 transfers spread over the 3 DMA-capable
            # engine queues (sync/scalar/gpsimd, ~140GB/s each) so the ramp
            # isn't serialized. First-needed data (xT halves, pair-0 q/k
            # slices) leads each queue; wv/wo/the remaining c-slices trail.
            xT = pp.tile([128, IH, KT, 512], BF16, name="xT", tag="xT")
            wq = pp.tile([128, NPAIR, KT, 128], BF16, name="wq", tag="wq")
            wk = pp.tile([128, NPAIR, KT, 128], BF16, name="wk", tag="wk")
            wv = pp.tile([128, KT, D], BF16, name="wv", tag="wv")
            wo = pp.tile([128, KT, D], BF16, name="wo", tag="wo")
            warm = pp.tile([128, 512], BF16, name="warm", tag="warm")
            CSL = KT * 128  # one c-slice of wq/wk per partition row
            nc.sync.dma_start(out=xT[:, 0], in_=xT_ext[:, : KT * 512])
            nc.gpsimd.dma_start(out=xT[:, 1], in_=xT_ext[:, KT * 512 :])
            nc.scalar.dma_start(out=wq[:, 0], in_=wq_ext[:, :CSL])
            nc.sync.dma_start(out=wk[:, 0], in_=wk_ext[:, :CSL])
            # memset on the DVE: a gpsimd memset between SWDGE triggers costs
            # a ~10us dge_drain on the gpsimd queue
            nc.vector.memset(warm, 0.0)
            nc.scalar.dma_start(out=wq[:, 1:], in_=wq_ext[:, CSL:])
            nc.sync.dma_start(out=wv, in_=wv_ext[:, :])
            nc.scalar.dma_start(out=wk[:, 1:], in_=wk_ext[:, CSL:])
            nc.gpsimd.dma_start(out=wo, in_=wo_ext[:, :])

            # PE warm-up during the DMA window: ~11 cold matmuls ~= 4.7us of
            # sustained PE activity (DMA data lands ~5.5us after trigger, so
            # the warmups bridge the whole wait) flips the HAM to 2.4GHz
            # right as the first real projection matmuls become ready
            for w_i in range(11):
                wps = ps_d.tile([128, 512], F32, name=f"ps_d{w_i % 2}", tag=f"ps_d{w_i % 2}")
                nc.tensor.matmul(out=wps, lhsT=warm[:, :128], rhs=warm,
                                 start=True, stop=True)

            # persistent activation storage
            qT = [pp.tile([128, P], BF16, name=f"qT{c}", tag=f"qT{c}") for c in range(NPAIR)]
            kTt = [pp.tile([128, P], BF16, name=f"kT{c}", tag=f"kT{c}") for c in range(NPAIR)]
            vt = [pp.tile([128, D], BF16, name=f"v{p}", tag=f"v{p}") for p in range(PT)]
            aoT = [pp.tile([128, P], BF16, name=f"aoT{c}", tag=f"aoT{c}") for c in range(NPAIR)]

            def proj_qk(w, ct, dst):
                """dst [128,P] = (x @ W)^T c-slice. Yields per i-half."""
                for ih in range(IH):
                    ps = ps_main.tile([128, 512], F32, name="ps_main", tag="ps_main")
                    for k in range(KT):
                        nc.tensor.matmul(
                            out=ps,
                            lhsT=w[:, ct, k, :],
                            rhs=xT[:, ih, k, :],
                            start=(k == 0), stop=(k == KT - 1),
                        )
                    nc.vector.tensor_copy(dst[:, ih * 512:(ih + 1) * 512], ps)
                    yield

            def proj_v(pt):
                """vt[pt] [128, D] = x p-tile @ W_v (bf16). Yields once."""
                ps = ps_main.tile([128, 512], F32, name="ps_main", tag="ps_main")
                for k in range(KT):
                    nc.tensor.matmul(
                        out=ps,
                        lhsT=xT[:, pt // 4, k, (pt % 4) * 128:(pt % 4) * 128 + 128],
                        rhs=wv[:, k, :],
                        start=(k == 0), stop=(k == KT - 1),
                    )
                nc.vector.tensor_copy(vt[pt], ps)
                yield

            pair_data = {}

            def pe_filler(n):
                """n dummy N=512 matmuls into the (idle) ps_main pool.

                The HAM re-throttles the PE to 1.2GHz when a ~3.4us activity
                window sees too much idle (observed threshold: phases at
                <=55% PE-busy go cold and stay cold; >=74% stay at 2.4GHz).
                The attention-only phases are elementwise-bound with the PE
                at ~37-55%, so they get padded with dummy matmuls.
                """
                for _ in range(n):
                    wps = ps_main.tile([128, 512], F32, name="ps_main", tag="ps_main")
                    nc.tensor.matmul(out=wps, lhsT=warm[:, :128], rhs=warm,
                                     start=True, stop=True)

            def attn_pair(pr, dummies=0):
                """dots + exp + row sums for head pair pr. Yields per (jt, h).

                Most tiles: ScalarE Exp with accum_out sums. DVE_TILES:
                Schraudolph fast-exp on DVE + identity-with-accum sums.
                After both heads of a j-tile are done, the reciprocal and
                V-row scaling for that j-tile are emitted immediately, so
                the AV contraction can start before the whole pair's exp
                stream finishes (no per-pair softmax-sum barrier).
                """
                a_t = [[ap_.tile([128, P], BF16, name=f"a{h}_{jt}", tag=f"a{h}_{jt}")
                        for jt in range(PT)] for h in range(2)]
                sums = sp.tile([128, 2, PT], F32, name="sums", tag="sums")
                rr = sp.tile([128, 2, PT], F32, name="recip", tag="recip")
                vp = [vpp.tile([128, 128], BF16, name=f"vp{jt}", tag=f"vp{jt}")
                      for jt in range(PT)]
                pair_data[pr] = (a_t, vp)
                for jt in range(PT):
                    for h in range(2):
                        hp = slice(h * 64, (h + 1) * 64)
                        ps = ps_c.tile([128, P], F32, name="ps_c", tag="ps_c")
                        for ih in range(IH):
                            nc.tensor.matmul(
                                out=ps[:, ih * 512:(ih + 1) * 512],
                                lhsT=kTt[pr][hp, jt * 128:(jt + 1) * 128],
                                rhs=qT[pr][hp, ih * 512:(ih + 1) * 512],
                                start=True, stop=True,
                                tile_position=(h * 64, 0),
                            )
                        if (jt, h) in DVE_TILES:
                            nc.vector.tensor_scalar(
                                out=a_t[h][jt].bitcast(I16),
                                in0=ps,
                                scalar1=SEXP_A,
                                scalar2=SEXP_B,
                                op0=mybir.AluOpType.mult,
                                op1=mybir.AluOpType.add,
                            )
                            nc.vector.tensor_scalar(
                                out=a_t[h][jt],
                                in0=a_t[h][jt],
                                scalar1=1.0,
                                scalar2=None,
                                op0=mybir.AluOpType.mult,
                                op1=mybir.AluOpType.add,
                                accum_out=sums[:, h, jt:jt + 1],
                            )
                        else:
                            nc.scalar.activation(
                                out=a_t[h][jt],
                                in_=ps,
                                func=mybir.ActivationFunctionType.Exp,
                                accum_out=sums[:, h, jt:jt + 1],
                            )
                        if h == 1:
                            nc.vector.reciprocal(
                                rr[:, :, jt:jt + 1], sums[:, :, jt:jt + 1]
                            )
                            for hh in range(2):
                                hc = (2 * pr + hh) * 64
                                # NOTE: keep this on the DVE — gpsimd takes
                                # ~1.15us per op (Q7 dispatch overhead) and
                                # this sits on the critical path to AV
                                nc.vector.tensor_scalar_mul(
                                    vp[jt][:, hh * 64:(hh + 1) * 64],
                                    vt[jt][:, hc:hc + 64],
                                    rr[:, hh, jt:jt + 1],
                                )
                            # dummies after both heads' dots so they never
                            # delay the exp stream
                            pe_filler(dummies)
                        yield

            def attn_av_ih(pr, ih):
                """contract A^T with V' for one i-half of pair pr -> aoT[pr].

                Both heads accumulate into one [128,512] psum tile (disjoint
                partition halves), so the drain is a single CAST.
                """
                a_t, vp = pair_data[pr]
                psd = ps_d.tile([128, 512], F32,
                                name=f"ps_d{ih % 2}", tag=f"ps_d{ih % 2}")
                for jt in range(PT):
                    for h in range(2):
                        nc.tensor.matmul(
                            out=psd[h * 64:(h + 1) * 64, :],
                            lhsT=vp[jt][:, h * 64:(h + 1) * 64],
                            rhs=a_t[h][jt][:, ih * 512:(ih + 1) * 512],
                            start=(jt == 0), stop=(jt == PT - 1),
                            tile_position=(0, h * 64),
                            skip_group_check=True,
                        )
                    if jt % 2 == 1:
                        yield
                nc.vector.tensor_copy(
                    aoT[pr][:, ih * 512:(ih + 1) * 512], psd
                )
                yield

            def attn_av(pr):
                yield from attn_av_ih(pr, 0)
                yield from attn_av_ih(pr, 1)

            def chain(*gens):
                for g in gens:
                    yield from g

            def interleave(main, filler, ms=2, fs=2, filler_first=False):
                """Emit ms units of main, then fs units of filler, repeating.

                filler_first puts the filler (usually the previous pair's AV,
                whose dependencies are already satisfied) ahead of the new
                pair's dots in the in-order PE queue each round.
                """
                order = ((filler, fs), (main, ms)) if filler_first \
                    else ((main, ms), (filler, fs))
                while True:
                    done = 0
                    for g, n in order:
                        try:
                            for _ in range(n):
                                next(g)
                        except StopIteration:
                            done += 1
                    if done == 2:
                        return

            def out_proj(pts):
                for pt in pts:
                    ps = ps_main.tile([128, 512], F32, name="ps_main", tag="ps_main")
                    for ct in range(KT):
                        nc.tensor.matmul(
                            out=ps,
                            lhsT=aoT[ct][:, pt * 128:(pt + 1) * 128],
                            rhs=wo[:, ct, :],
                            start=(ct == 0), stop=(ct == KT - 1),
                        )
                    ot = op_.tile([128, 512], BF16, name="osb", tag="osb")
                    nc.vector.tensor_copy(ot, ps)
                    eng = nc.sync if pt % 2 == 0 else nc.scalar
                    eng.dma_start(out=out_ext[pt * 128:(pt + 1) * 128, :], in_=ot)
                    yield

            for g in chain(proj_qk(wq, 0, qT[0]), proj_qk(wk, 0, kTt[0])):
                pass
            # filler_first so each proj_v(jt) is emitted before the pair-0
            # vp-scaling unit that reads vt[jt]. Pair phases 1-2 are
            # PE-starved (elementwise-bound), so they get dummy-matmul
            # padding to keep the HAM from re-throttling the PE clock.
            # ms=2/fs=1: two dots tiles per filler unit. The exp span is
            # production-rate-limited at 1:1 (one tile per ~1.7us round vs
            # ScalarE's 1.4us/tile consumption); a production surplus lets
            # the exp stream run back-to-back and the filler drains inside
            # the PE's psum-free stall slack.
            interleave(
                attn_pair(0),
                chain(*[proj_v(pt) for pt in range(PT)],
                      proj_qk(wq, 1, qT[1]), proj_qk(wk, 1, kTt[1]),
                      proj_qk(wq, 2, qT[2]), proj_qk(wk, 2, kTt[2])),
                ms=2, fs=2, filler_first=True,
            )
            interleave(
                attn_pair(1, dummies=2),
                chain(attn_av(0),
                      proj_qk(wq, 3, qT[3]), proj_qk(wk, 3, kTt[3])),
                ms=2, fs=2, filler_first=True,
            )
            interleave(attn_pair(2, dummies=4), attn_av(1),
                       ms=2, fs=2, filler_first=True)
            # pair 3: AV(2), the pair-3 AV halves and the out-projection all
            # pipeline into the exp-supply rounds. ms=3/fs=2 keeps every
            # pair-3 AV j-tile behind its dots in the in-order PE queue
            # (emitting an AV j-tile before its dots would deadlock the PE).
            interleave(
                attn_pair(3),
                chain(attn_av(2),
                      attn_av_ih(3, 0),
                      out_proj(range(4)),
                      attn_av_ih(3, 1),
                      out_proj(range(4, PT))),
                ms=3, fs=2, filler_first=True,
            )


    nc.finalize()
    return nc


_NC = None


def _get_nc():
    global _NC
    if _NC is None:
        _NC = build()
    return _NC


def run(x, W_qkv, W_out, b_out, trace=False, tmpdir=None):
    import ml_dtypes

    x = np.asarray(x, dtype=np.float32)
    W_qkv = np.asarray(W_qkv, dtype=np.float32)
    W_out = np.asarray(W_out, dtype=np.float32)
    b_out = np.asarray(b_out, dtype=np.float32)

    bf = ml_dtypes.bfloat16

    def pack(a):
        # [KT*128, W] -> [128, KT*W] (k-tile-major per partition row)
        w = a.shape[1]
        return np.ascontiguousarray(
            a.reshape(KT, 128, w).transpose(1, 0, 2).reshape(128, KT * w)
        ).astype(bf)

    def pack_x(a):
        # a = x[b].T [D, P] -> [128, IH*KT*512] (i-half-major, then k-tile)
        t = a.reshape(KT, 128, IH, 512)
        return np.ascontiguousarray(
            t.transpose(1, 2, 0, 3).reshape(128, IH * KT * 512)
        ).astype(bf)

    def pack_c(a):
        # [KT*128, D] -> [128, NPAIR*KT*128] (c-slice-major, then k-tile)
        t = a.reshape(KT, 128, H // 2, 128)
        return np.ascontiguousarray(
            t.transpose(1, 2, 0, 3).reshape(128, (H // 2) * KT * 128)
        ).astype(bf)

    wq_h = pack_c(W_qkv[:, :D] * np.float32(SCALE))
    wk_h = pack_c(W_qkv[:, D:2 * D])
    wv_h = pack(W_qkv[:, 2 * D:])
    wo_h = pack(W_out)
    in_maps = [
        {
            "xT": pack_x(np.ascontiguousarray(x[b].T)),
            "wq": wq_h, "wk": wk_h, "wv": wv_h, "wo": wo_h,
        }
        for b in range(NCORES)
    ]
    nc = _get_nc()
    res = run_bass_kernel_spmd(
        nc, in_maps, core_ids=list(range(NCORES)), trace=trace, tmpdir=tmpdir
    )
    out = np.stack(
        [res.results[b]["out"].astype(np.float32) for b in range(NCORES)], axis=0
    )
    out = out + b_out[None, None, :]
    return out.astype(np.float32), res


def kernel(x, W_qkv, W_out, b_out):
    out, _ = run(x, W_qkv, W_out, b_out, trace=False)
    return out

